# revision 1
# baseline (speedup 1.0000x reference)
"""DGL capsule routing layer on 8 trn2 NeuronCores (Bass/Tile).

Math: for routing_num iterations,
    c = softmax(b, axis=out)                        # b0 = 0
    s = einsum('io,iof->of', c, uh)
    v = squash(s)
    b = b + einsum('iof,of->io', uh, v)
Output: final v [OUT, F].

Key identity: b_t = uh . (v_1 + ... + v_t)  (b is linear in uh), so b is
never materialized across iterations; each iteration is one streaming pass
over uh with w_t = cumulative sum of v's:
    pass t: b = sum_f uh[i,o,f]*w[o,f]; e = exp(b); r_i = 1/sum_o e
            s[o,f] = sum_i r_i * e[i,o] * uh[i,o,f]   (partial per core)
            AllReduce(s); v = squash(s); w += v
Pass 1 has c uniform (=1/OUT) so it is a pure PE pass.

Sharding: i (in_nodes) split across 8 cores, 512 rows each (4 blocks of
128 partitions). Engine plan per 2048-wide o-f chunk (passes >= 2):
  GpSimd: tm = uh * w_bcast        (2-input mul; DVE TT never contends)
  DVE:    b-slice = segsum_f(tm);  p = e * uh (e broadcast over f)
  ACT:    e = exp(b) with fused denominator accum; psum flushes
  PE:     s-partial = sum_i rinv[i]*p[i,:] as 4x N=512 matmuls with
          rinv as the 1-column stationary operand -> psum [1,2048]
The per-block s partials go straight to DRAM [4,16384]; the AllReduce sums
over cores, and the cheap cross-block sum happens after the AR in the
partition-spread [128,128] layout (3 DVE adds).
"""

import numpy as np
from contextlib import ExitStack

import concourse.bass as bass
import concourse.mybir as mybir
import concourse.tile as tile
from concourse import bacc
from concourse import bass_utils

F32 = mybir.dt.float32
AX = mybir.AxisListType
AF = mybir.ActivationFunctionType

IN_NODES, OUT_NODES, F_SIZE = 4096, 1024, 16
CORES = 8
I_LOC = IN_NODES // CORES          # 512 in-nodes per core
ROW = OUT_NODES * F_SIZE           # 16384 floats per in-node row
P = 128
NBLK = I_LOC // P                  # 4 i-blocks per core
QT = 4096                          # streamed quarter width (elems/partition)
NQT = ROW // QT                    # 4 quarters per block
CH = 2048                          # chunk/piece width (elems/partition)
NCH_Q = QT // CH                   # 2 chunks per quarter
NMM = CH // 512                    # 4 matmuls per piece
F32R_MM = True                     # fast-path fp32 matmuls (1 cyc/row)
MM_DT = mybir.dt.float32r if F32R_MM else F32


def _body(nc, tc, uh, v_out, R, rg):
    uh_t = uh.rearrange("(n p) r -> n p r", p=P)   # [NBLK, 128, 16384]

    with ExitStack() as ctx:
        io = ctx.enter_context(tc.tile_pool(name="io", bufs=4))
        work = ctx.enter_context(tc.tile_pool(name="work", bufs=4))
        small = ctx.enter_context(tc.tile_pool(name="small", bufs=2))
        persist = ctx.enter_context(tc.tile_pool(name="persist", bufs=1))
        pspool = ctx.enter_context(tc.tile_pool(name="pspool", bufs=2, space="PSUM"))
        dram = ctx.enter_context(tc.tile_pool(name="dram", bufs=2, space="DRAM"))

        c0_f = persist.tile([P, 1], F32, name="c0_f")
        nc.vector.memset(c0_f, 1.0 / OUT_NODES)
        c0 = persist.tile([P, 1], MM_DT, name="c0")
        nc.vector.tensor_copy(c0, c0_f)
        w_sb = w_acc = None
        if R > 1:
            w_sb = persist.tile([P, ROW], F32, name="w_sb")
            w_acc = persist.tile([P, P], F32, name="w_acc")

        for t in range(1, R + 1):
            ar_in = dram.tile([NBLK, ROW], F32, tag="ar_in")
            for blk in range(NBLK):
                uts = []
                for q in range(NQT):
                    ut = io.tile([P, QT], F32, tag="ut")
                    nc.sync.dma_start(ut, uh_t[blk, :, q * QT:(q + 1) * QT])
                    uts.append(ut)
                if t == 1:
                    rinv = c0
                else:
                    b = small.tile([P, OUT_NODES], F32, tag="b")
                    for q in range(NQT):
                        for k in range(NCH_Q):
                            sl = slice(k * CH, (k + 1) * CH)
                            g0 = q * QT + k * CH
                            tm = work.tile([P, CH], F32, tag="tm")
                            # b-mul on GpSimd (concurrent with DVE TT/reduce)
                            nc.gpsimd.tensor_mul(
                                tm, uts[q][:, sl], w_sb[:, g0:g0 + CH])
                            o0 = g0 // F_SIZE
                            nc.vector.reduce_sum(
                                b[:, o0:o0 + CH // F_SIZE],
                                tm.rearrange("p (o f) -> p o f", f=F_SIZE),
                                axis=AX.X,
                            )
                    e = small.tile([P, OUT_NODES], F32, tag="e")
                    den = small.tile([P, 1], F32, tag="den")
                    nc.scalar.activation(e, b, AF.Exp, accum_out=den)
                    rinv_f = small.tile([P, 1], F32, tag="rinv_f")
                    nc.vector.reciprocal(rinv_f, den)
                    rinv = small.tile([P, 1], MM_DT, tag="rinv")
                    nc.vector.tensor_copy(rinv, rinv_f)
                for q in range(NQT):
                    for k in range(NCH_Q):
                        sl = slice(k * CH, (k + 1) * CH)
                        g0 = q * QT + k * CH
                        pt = work.tile([P, CH], MM_DT, tag="tm")
                        if t == 1:
                            # round to f32r on idle DVE (pass 1 only)
                            nc.vector.tensor_copy(pt, uts[q][:, sl])
                        else:
                            o0 = g0 // F_SIZE
                            och = CH // F_SIZE
                            nc.vector.tensor_mul(
                                pt.rearrange("p (o f) -> p o f", f=F_SIZE),
                                uts[q][:, sl].rearrange(
                                    "p (o f) -> p o f", f=F_SIZE),
                                e[:, o0:o0 + och][:, :, None].broadcast_to(
                                    [P, och, F_SIZE]),
                            )
                        ps = pspool.tile([1, CH], F32, tag="ps")
                        for c in range(NMM):
                            nc.tensor.matmul(
                                ps[:, c * 512:(c + 1) * 512],
                                rinv,
                                pt[:, c * 512:(c + 1) * 512],
                                start=True, stop=True,
                                skip_group_check=True,
                            )
                        fl = small.tile([1, CH], F32, tag="fl")
                        nc.scalar.copy(fl, ps)
                        nc.sync.dma_start(ar_in[blk, g0:g0 + CH], fl)
            ar_out = dram.tile([NBLK, ROW], F32, tag="ar_out")
            nc.gpsimd.collective_compute(
                "AllReduce", mybir.AluOpType.add, replica_groups=rg,
                ins=[ar_in.opt()], outs=[ar_out.opt()],
            )
            # s2[p,(j,f)] with o = p*8+j: sum the 4 block rows post-AR
            slds = []
            for blk in range(NBLK):
                sld = small.tile([P, P], F32, tag="sld", bufs=4)
                nc.sync.dma_start(
                    sld, ar_out[blk].rearrange("(p q) -> p q", p=P))
                slds.append(sld)
            s2 = small.tile([P, P], F32, tag="s2")
            nc.vector.tensor_add(s2, slds[0], slds[1])
            nc.vector.tensor_add(s2, s2, slds[2])
            nc.vector.tensor_add(s2, s2, slds[3])
            # squash: v = s * sqrt(sq)/(1+sq), sq = sum_f s^2
            ssq = small.tile([P, P], F32, tag="ssq")
            nc.vector.tensor_mul(ssq, s2, s2)
            sq = small.tile([P, 8], F32, tag="sq")
            nc.vector.reduce_sum(
                sq, ssq.rearrange("p (j f) -> p j f", f=F_SIZE), axis=AX.X)
            # sqrt via exp(0.5*ln(x)): stays in the exp/ln ACT table set
            lnq = small.tile([P, 8], F32, tag="lnq")
            nc.scalar.activation(lnq, sq, AF.Ln)
            y = small.tile([P, 8], F32, tag="y")
            nc.scalar.activation(y, lnq, AF.Exp, scale=0.5)
            # one Newton step: y <- 0.5*(y + sq/y)
            ry = small.tile([P, 8], F32, tag="ry")
            nc.vector.reciprocal(ry, y)
            t1 = small.tile([P, 8], F32, tag="t1")
            nc.vector.tensor_mul(t1, sq, ry)
            nc.vector.tensor_add(t1, t1, y)
            nc.vector.tensor_scalar_mul(t1, t1, 0.5)
            d1 = small.tile([P, 8], F32, tag="d1")
            nc.vector.tensor_scalar_add(d1, sq, 1.0)
            rd = small.tile([P, 8], F32, tag="rd")
            nc.vector.reciprocal(rd, d1)
            sc = small.tile([P, 8], F32, tag="sc")
            nc.vector.tensor_mul(sc, t1, rd)
            v_sb = small.tile([P, P], F32, tag="v_sb")
            nc.vector.tensor_mul(
                v_sb.rearrange("p (j f) -> p j f", f=F_SIZE),
                s2.rearrange("p (j f) -> p j f", f=F_SIZE),
                sc[:, :, None].broadcast_to([P, 8, F_SIZE]),
            )
            if t == R:
                nc.sync.dma_start(
                    v_out.rearrange("(p j) f -> p (j f)", j=8), v_sb)
            else:
                if t == 1:
                    nc.scalar.copy(w_acc, v_sb)
                else:
                    nc.vector.tensor_add(w_acc, w_acc, v_sb)
                # broadcast w to all partitions via DRAM round-trip:
                # w_acc[p,(j,f)] -> flat w_dram[o*16+f] -> [128, ROW] bcast
                w_dram = dram.tile([ROW], F32, tag="w_dram")
                nc.sync.dma_start(
                    w_dram.rearrange("(p q) -> p q", p=P), w_acc)
                wd_b = w_dram.unsqueeze(0)
                for j in range(8):
                    sl = slice(j * CH, (j + 1) * CH)
                    nc.sync.dma_start(
                        w_sb[:, sl],
                        wd_b[:, sl].broadcast_to([P, CH]))


def _build(routing_num: int):
    R = int(routing_num)
    assert R >= 1
    nc = bacc.Bacc(
        "TRN2", target_bir_lowering=False, debug=False, num_devices=CORES)
    uh = nc.dram_tensor("uh", [I_LOC, ROW], F32, kind="ExternalInput")
    v_out = nc.dram_tensor("v_out", [OUT_NODES, F_SIZE], F32,
                           kind="ExternalOutput")
    rg = [list(range(CORES))]
    with tile.TileContext(nc) as tc:
        _body(nc, tc, uh.ap(), v_out.ap(), R, rg)
    nc.compile()
    return nc


_CACHE: dict = {}


def _get_nc(routing_num: int):
    R = int(routing_num)
    if R not in _CACHE:
        _CACHE[R] = _build(R)
    return _CACHE[R]


def _shard(u_hat: np.ndarray):
    uh = np.ascontiguousarray(np.asarray(u_hat, dtype=np.float32))
    assert uh.shape == (IN_NODES * OUT_NODES, F_SIZE), uh.shape
    uh = uh.reshape(IN_NODES, ROW)
    return [
        {"uh": np.ascontiguousarray(uh[k * I_LOC:(k + 1) * I_LOC])}
        for k in range(CORES)
    ]


def run(u_hat, routing_num, trace=False):
    nc = _get_nc(routing_num)
    in_maps = _shard(u_hat)
    res = bass_utils.run_bass_kernel_spmd(
        nc, in_maps, core_ids=list(range(CORES)), trace=trace)
    return res


def kernel(u_hat, routing_num):
    res = run(u_hat, routing_num, trace=False)
    return np.asarray(res.results[0]["v_out"], dtype=np.float32)



# revision 4
# speedup vs baseline: 1.7172x; 1.7172x over previous
"""DGL capsule routing layer on 8 trn2 NeuronCores (Bass/Tile).

Math: for routing_num iterations,
    c = softmax(b, axis=out)                        # b0 = 0
    s = einsum('io,iof->of', c, uh)
    v = squash(s)
    b = b + einsum('iof,of->io', uh, v)
Output: final v [OUT, F].

Identity: b_t = uh . (v_1 + ... + v_{t-1}) = uh . w, so b is recomputed
from the cumulative w each pass instead of being materialized in DRAM.

Sharding: OUT_NODES split across 8 cores (128 o's per core); every core
holds ALL 4096 in-nodes of its o-slice. Softmax over o needs a cross-core
sum of the per-i denominators only: AllReduce of [128,16] f32 (8 KiB) per
half-pass, hidden under compute. s/v/squash are then fully core-local and
the final output is a host-side concat of the 8 v-shards.

Memory plan: the per-core shard (4096 x 128 x 16 f32 = 32 MiB) is read
from HBM once (pass 1) and kept resident in SBUF as bf16 (16 MiB) in
f-major layout [i-part, (f,o)]:
  - f-major makes every hot DVE op contiguous step-1 bf16 => 2x DVE mode
    (tensor_reduce only runs at 1x, so the f-sum is a 4-level TT tree).
  - per i-block (128 i's): tm = uh*w_bcast (TT); tree over f-halves -> b
    [128,128] f32; ACT exp with accum_out -> e (bf16) + den column.
  - p = e (bcast over f) * uh (TT 2x); PE matmul with rinv[i] as the
    1-column bf16 stationary accumulates s [1,2048] over all 32 blocks
    in a single PSUM bank (one flush per pass).
  - squash runs partition-spread [128o,16f] via a tiny DRAM round trip;
    w broadcast back to all partitions via DMA-broadcast + cast.
"""

import numpy as np
from contextlib import ExitStack

import concourse.bass as bass
import concourse.mybir as mybir
import concourse.tile as tile
from concourse import bacc
from concourse import bass_utils

F32 = mybir.dt.float32
BF16 = mybir.dt.bfloat16
AX = mybir.AxisListType
AF = mybir.ActivationFunctionType

IN_NODES, OUT_NODES, F_SIZE = 4096, 1024, 16
CORES = 8
O_LOC = OUT_NODES // CORES         # 128 out-nodes per core
ROWL = O_LOC * F_SIZE              # 2048 elems per in-node row (local)
P = 128
NBLK = IN_NODES // P               # 32 i-blocks per core
NMM = ROWL // 512                  # 4 matmuls per block (psum bank = 512 f32)
HALF = NBLK // 2                   # 16 blocks per AR half


def _body(nc, tc, uh, v_out, R, rg):
    uh_t = uh.rearrange("(n p) r -> n p r", p=P)   # [32, 128, 2048]

    with ExitStack() as ctx:
        stage = ctx.enter_context(tc.tile_pool(name="stage", bufs=2))
        work = ctx.enter_context(tc.tile_pool(name="work", bufs=2))
        ppool = ctx.enter_context(tc.tile_pool(name="ppool", bufs=2))
        small = ctx.enter_context(tc.tile_pool(name="small", bufs=2))
        sflush = ctx.enter_context(tc.tile_pool(name="sflush", bufs=1))
        persist = ctx.enter_context(tc.tile_pool(name="persist", bufs=1))
        pspool = ctx.enter_context(tc.tile_pool(name="pspool", bufs=2, space="PSUM"))
        dram = ctx.enter_context(tc.tile_pool(name="dram", bufs=2, space="DRAM"))

        res = persist.tile([P, NBLK * ROWL], BF16, name="res")
        e_all = persist.tile([P, NBLK * P], BF16, name="e_all")
        den = persist.tile([P, NBLK], F32, name="den")
        rinv16 = persist.tile([P, NBLK], BF16, name="rinv16")
        w_bcast = persist.tile([P, ROWL], BF16, name="w_bcast")
        w2 = persist.tile([P, F_SIZE], F32, name="w2")

        c0f = persist.tile([P, 1], F32, name="c0f")
        nc.vector.memset(c0f, 1.0 / OUT_NODES)
        c0 = persist.tile([P, 1], BF16, name="c0")
        nc.vector.tensor_copy(c0, c0f)

        for t in range(1, R + 1):
            s_ps = pspool.tile([1, ROWL], F32, tag="s_ps")
            if t == 1:
                # stream from HBM, cast+shuffle (o,f)->(f,o) into residency,
                # and run the uniform-coupling s-matmuls off the fresh tiles
                for blk in range(NBLK):
                    st = stage.tile([P, ROWL], F32, tag="st")
                    nc.sync.dma_start(st, uh_t[blk])
                    rs = res[:, blk * ROWL:(blk + 1) * ROWL]
                    nc.vector.tensor_copy(
                        rs.rearrange("p (f o) -> p f o", o=O_LOC),
                        st.rearrange("p (o f) -> p f o", f=F_SIZE),
                    )
                    for c in range(NMM):
                        nc.tensor.matmul(
                            s_ps[:, c * 512:(c + 1) * 512],
                            c0,
                            rs[:, c * 512:(c + 1) * 512],
                            start=(blk == 0), stop=(blk == NBLK - 1),
                            skip_group_check=True,
                        )
            else:
                # b-phase: b = sum_f uh*w, e = exp(b), den-accum; AR per half
                for h in range(2):
                    for blk in range(h * HALF, (h + 1) * HALF):
                        rs = res[:, blk * ROWL:(blk + 1) * ROWL]
                        tm = work.tile([P, ROWL], BF16, tag="tm")
                        nc.vector.tensor_mul(tm, rs, w_bcast)
                        l1 = work.tile([P, 1024], BF16, tag="l1")
                        nc.vector.tensor_add(l1, tm[:, :1024], tm[:, 1024:])
                        l2 = work.tile([P, 512], BF16, tag="l2")
                        nc.vector.tensor_add(l2, l1[:, :512], l1[:, 512:])
                        l3 = work.tile([P, 256], BF16, tag="l3")
                        nc.vector.tensor_add(l3, l2[:, :256], l2[:, 256:])
                        bb = work.tile([P, P], F32, tag="bb")
                        nc.vector.tensor_add(bb, l3[:, :128], l3[:, 128:])
                        nc.scalar.activation(
                            e_all[:, blk * P:(blk + 1) * P], bb, AF.Exp,
                            accum_out=den[:, blk:blk + 1])
                    ar_in = dram.tile([P, HALF], F32, tag="ar_in")
                    nc.sync.dma_start(
                        ar_in, den[:, h * HALF:(h + 1) * HALF])
                    ar_out = dram.tile([P, HALF], F32, tag="ar_out")
                    nc.gpsimd.collective_compute(
                        "AllReduce", mybir.AluOpType.add, replica_groups=rg,
                        ins=[ar_in.opt()], outs=[ar_out.opt()],
                    )
                    deng = small.tile([P, HALF], F32, tag="deng")
                    nc.sync.dma_start(deng, ar_out)
                    rf = small.tile([P, HALF], F32, tag="rf")
                    nc.vector.reciprocal(rf, deng)
                    nc.vector.tensor_copy(
                        rinv16[:, h * HALF:(h + 1) * HALF], rf)
                # p-phase: p = e (bcast over f) * uh; s += rinv^T @ p
                for blk in range(NBLK):
                    rs = res[:, blk * ROWL:(blk + 1) * ROWL]
                    p = ppool.tile([P, ROWL], BF16, tag="p")
                    e_sl = e_all[:, blk * P:(blk + 1) * P]
                    nc.vector.tensor_mul(
                        p.rearrange("p (f o) -> p f o", o=O_LOC),
                        rs.rearrange("p (f o) -> p f o", o=O_LOC),
                        e_sl[:, None, :].broadcast_to([P, F_SIZE, O_LOC]),
                    )
                    for c in range(NMM):
                        nc.tensor.matmul(
                            s_ps[:, c * 512:(c + 1) * 512],
                            rinv16[:, blk:blk + 1],
                            p[:, c * 512:(c + 1) * 512],
                            start=(blk == 0), stop=(blk == NBLK - 1),
                            skip_group_check=True,
                        )

            # tail: flush s, squash partition-spread, update + broadcast w
            s_sb = sflush.tile([1, ROWL], F32, tag="s_sb")
            nc.scalar.copy(s_sb, s_ps)
            s_dram = dram.tile([ROWL], F32, tag="s_dram")
            nc.sync.dma_start(s_dram.unsqueeze(0), s_sb)
            s2 = small.tile([P, F_SIZE], F32, tag="s2")
            nc.sync.dma_start(s2, s_dram.rearrange("(f o) -> o f", o=O_LOC))
            # squash: v = s * sqrt(sq)/(1+sq), sq = sum_f s^2
            ssq = small.tile([P, F_SIZE], F32, tag="ssq")
            nc.vector.tensor_mul(ssq, s2, s2)
            sq = small.tile([P, 1], F32, tag="sq")
            nc.vector.reduce_sum(sq, ssq, axis=AX.X)
            # sqrt via exp(0.5*ln(x)) + one Newton step (exp/ln table set)
            lnq = small.tile([P, 1], F32, tag="lnq")
            nc.scalar.activation(lnq, sq, AF.Ln)
            y = small.tile([P, 1], F32, tag="y")
            nc.scalar.activation(y, lnq, AF.Exp, scale=0.5)
            ry = small.tile([P, 1], F32, tag="ry")
            nc.vector.reciprocal(ry, y)
            t1 = small.tile([P, 1], F32, tag="t1")
            nc.vector.tensor_mul(t1, sq, ry)
            nc.vector.tensor_add(t1, t1, y)
            nc.vector.tensor_scalar_mul(t1, t1, 0.5)
            d1 = small.tile([P, 1], F32, tag="d1")
            nc.vector.tensor_scalar_add(d1, sq, 1.0)
            rd = small.tile([P, 1], F32, tag="rd")
            nc.vector.reciprocal(rd, d1)
            sc = small.tile([P, 1], F32, tag="sc")
            nc.vector.tensor_mul(sc, t1, rd)
            v2 = small.tile([P, F_SIZE], F32, tag="v2")
            nc.vector.tensor_mul(v2, s2, sc.broadcast_to([P, F_SIZE]))
            if t == R:
                nc.sync.dma_start(v_out, v2)
            else:
                if t == 1:
                    nc.scalar.copy(w2, v2)
                else:
                    nc.vector.tensor_add(w2, w2, v2)
                w_dram = dram.tile([ROWL], F32, tag="w_dram")
                nc.sync.dma_start(
                    w_dram.rearrange("(f o) -> o f", o=O_LOC), w2)
                wbf = stage.tile([P, ROWL], F32, tag="st")
                nc.sync.dma_start(
                    wbf, w_dram.unsqueeze(0).broadcast_to([P, ROWL]))
                nc.vector.tensor_copy(w_bcast, wbf)


def _build(routing_num: int):
    R = int(routing_num)
    assert R >= 1
    nc = bacc.Bacc(
        "TRN2", target_bir_lowering=False, debug=False, num_devices=CORES)
    uh = nc.dram_tensor("uh", [IN_NODES, ROWL], F32, kind="ExternalInput")
    v_out = nc.dram_tensor("v_out", [O_LOC, F_SIZE], F32,
                           kind="ExternalOutput")
    rg = [list(range(CORES))]
    with tile.TileContext(nc) as tc:
        _body(nc, tc, uh.ap(), v_out.ap(), R, rg)
    nc.compile()
    return nc


_CACHE: dict = {}


def _get_nc(routing_num: int):
    R = int(routing_num)
    if R not in _CACHE:
        _CACHE[R] = _build(R)
    return _CACHE[R]


def _shard(u_hat: np.ndarray):
    uh = np.asarray(u_hat, dtype=np.float32)
    assert uh.shape == (IN_NODES * OUT_NODES, F_SIZE), uh.shape
    uh3 = uh.reshape(IN_NODES, OUT_NODES, F_SIZE)
    return [
        {"uh": np.ascontiguousarray(
            uh3[:, k * O_LOC:(k + 1) * O_LOC, :]).reshape(IN_NODES, ROWL)}
        for k in range(CORES)
    ]


def run(u_hat, routing_num, trace=False):
    nc = _get_nc(routing_num)
    in_maps = _shard(u_hat)
    res = bass_utils.run_bass_kernel_spmd(
        nc, in_maps, core_ids=list(range(CORES)), trace=trace)
    return res


def gather(res) -> np.ndarray:
    return np.concatenate(
        [np.asarray(res.results[c]["v_out"], dtype=np.float32)
         for c in range(CORES)], axis=0)


def kernel(u_hat, routing_num):
    res = run(u_hat, routing_num, trace=False)
    return gather(res)
